# revision 40
# baseline (speedup 1.0000x reference)
"""Fused CIN-layer kernel for Trainium2 (8 NeuronCores, batch data-parallel).

True reference semantics (derived from the row-major .view + strided conv):
  out[b, n, c*32+t] = sum_{i<32, y<32} W[n,i,y] * x0[b,t,2i+c] * xk[b,y,2i+c] + bias[n]
where c in {0,1} is the f-parity and i indexes f-pairs.

Per core (128 batches, bc = b_local*2 + c in [0,256)):
  warmup:  ~14 dummy matmuls on a zeroed tile while input DMA streams, so
           the PE HAM clock-gate reaches 2.4 GHz before the real work.
  stage1:  row-tiled quads: for quad q, 4 concurrent K=32 matmuls (one per
           i=4q+di, tile_position=(32*di,0)) with lhsT=W_i [32y,64n] and
           rhs=XkS_i [32y,256bc], each into its own PSUM bank of a
           [64, 2048] quad tile.  3-way ACT/DVE/GPSIMD evac into
           gsb[n, bc*32+i] (fp16).
  transpose (PE): per J (4 bc's): Gt_J[(j,i), n] from gsb[n, (j,i)] via PE
           transpose (fp16 PSUM out), DVE-evacuated to SBUF.
  stage2:  out_J[(j,t), n] = x0bd_J^T @ Gt_J where x0bd is a block-diagonal
           x0 tile built ON DEVICE (memset + 4 strided scatter DMAs of the
           compact 512KB x0), loaded as stationary with FWL.
  fp16 output DMA per J8 chunk; host adds bias + final reshape in fp32.
"""

import numpy as np

BS, T, Y, F, NF = 1024, 32, 32, 64, 64
NCORES = 8
BPC = BS // NCORES      # 128 batches per core
NBC = BPC * 2           # 256 (b,c) pairs per core
NG = NBC // 4           # 64 groups of 4
NI = 32                 # f-pair index
NWARM = 8               # HAM warmup matmuls

_cached = {}


def _build_bass():
    import concourse.bass as bass
    import concourse.mybir as mybir
    from concourse import bacc
    from concourse.tile import TileContext

    F16 = mybir.dt.float16
    F32 = mybir.dt.float32

    nc = bacc.Bacc()
    # comba[di*32+y | (n,i)-rows, :]: cols 0:512   w4[., q*64+n]  = W[n,4q+di,y]
    #                                cols 512:1536 xks half0 [., q*256+bc]
    #                                cols 1536:1600 iden (rows 0:64)
    comba = nc.dram_tensor("comba", [128, 1664], F16, kind="ExternalInput")
    # xksb: xks half1 (quads 4..7)
    xksb = nc.dram_tensor("xksb", [128, 4 * NBC], F16, kind="ExternalInput")
    # x0a[(j,i), J*128 + j2*32 + t] = delta(j,j2) * x0bc[4J+j, i, t]
    x0a = nc.dram_tensor("x0a", [128, NG * 128], F16, kind="ExternalInput")
    # outd[(j,t), J*64+n] fp16
    outd = nc.dram_tensor("outd", [128, NG * NF], F16, kind="ExternalOutput")

    with TileContext(nc) as tc:
        with (
            tc.tile_pool(name="const", bufs=1) as cpool,
            tc.tile_pool(name="sb", bufs=1) as spool,
        ):
            wdum = cpool.tile([128, 384], F16)
            nc.vector.memset(wdum[:, :], 0)
            # dependency-free scalar op so ACT_TABLE_LOAD happens during the
            # preamble instead of stalling the first stage-1 evacuation
            scratch = cpool.tile([64, 8], F16)
            nc.scalar.copy(scratch[:, :], wdum[0:64, 0:8])

            # single DMA for everything stage-1 needs (w4 + xks half0 + iden)
            comba_sb = cpool.tile([128, 1664], F16)
            nc.sync.dma_start(out=comba_sb, in_=comba[:, :])
            xksb_sb = cpool.tile([128, 4 * NBC], F16)
            nc.sync.dma_start(out=xksb_sb, in_=xksb[:, :])

            def w4_ap(di, q):
                return comba_sb[32 * di:32 * (di + 1), NF * q:NF * (q + 1)]

            def xks_ap(di, q):
                qc = q % 4
                if q < 4:
                    return comba_sb[32 * di:32 * (di + 1),
                                    512 + NBC * qc:512 + NBC * (qc + 1)]
                return xksb_sb[32 * di:32 * (di + 1), NBC * qc:NBC * (qc + 1)]

            id128 = comba_sb[:, 1536:1664]
            # x0a in two tiles so early s2 rounds aren't gated on the full 2MB
            x0a_sb = [cpool.tile([128, NG * 64], F16, name=f"x0a_sb{h}") for h in range(2)]
            for h in range(2):
                nc.sync.dma_start(out=x0a_sb[h],
                                  in_=x0a[:, NG * 64 * h:NG * 64 * (h + 1)])

            # gsb2[(g,n), m*128 + j*32 + i] = G[n, bc=8m+4g+j, i]
            gsb = spool.tile([128, NBC * NI // 2], F16)
            osb = spool.tile([128, NG * NF], F16)    # out[(j,t), J*64+n]

            # HAM warmup: harmless matmuls on zeros while inputs stream in
            with tc.tile_pool(name="warm", bufs=1, space="PSUM") as wpool:
                wt = wpool.tile([128, 256], F32)
                for _ in range(NWARM):
                    nc.tensor.matmul(wt[:, :], wdum[:, :128], wdum[:, 128:384],
                                     start=True, stop=True)

            # stage 1: per quad q, 8 matmuls (4 row-tiled i's x 2 J-parities).
            # Parity g goes to output partitions [64g, 64g+64) (column tile
            # position 64g) so the psum quad uses all 128 partitions and the
            # evacuations run full-lane.  rhs selects bc's of parity g
            # (stride-8 blocks of 4).
            with tc.tile_pool(name="gq", bufs=2, space="PSUM") as gqpool:
                for q in range(8):
                    gq = gqpool.tile([128, 2048], F32, tag="gq")
                    for g in range(2):
                        for di in range(4):
                            rhs = xks_ap(di, q).rearrange(
                                "p (m gj) -> p m gj", m=NI, gj=8)[:, :, 4 * g:4 * g + 4]
                            nc.tensor.matmul(
                                gq[64 * g:64 * (g + 1), 512 * di:512 * di + 128],
                                w4_ap(di, q),
                                rhs,
                                start=True, stop=True,
                                tile_position=(32 * di, 64 * g),
                            )
                    # evac: gsb2[(g,n), m*128 + i*4 + j] -- m-major keeps each
                    # transpose input one contiguous 128-col run, i*4+j makes
                    # each i-pair evac write 16B bursts
                    for h in range(2):
                        src = gq[:, :].rearrange(
                            "p (di mm j) -> p mm di j",
                            di=4, mm=128, j=4)[:, 0:NI, 2 * h:2 * h + 2, :]
                        dst = gsb[:, :].rearrange(
                            "p (m i j) -> p m i j",
                            m=NI, i=NI, j=4)[:, :, 4 * q + 2 * h:4 * q + 2 * h + 2, :]
                        if h == 0:
                            nc.vector.tensor_copy(dst, src)
                        else:
                            nc.scalar.copy(dst, src)

            # transpose + stage2, software-pipelined per J8 round
            with (
                tc.tile_pool(name="gt", bufs=3, space="PSUM") as gtpool,
                tc.tile_pool(name="po", bufs=2, space="PSUM") as popool,
                tc.tile_pool(name="gts", bufs=3) as gtspool,
            ):
                gts_tiles = {}

                def do_tr(J8):
                    gt4 = gtpool.tile([128, 512], F16, tag="gt8")
                    for k in range(4):
                        m = 4 * J8 + k
                        nc.tensor.transpose(
                            gt4[:, 128 * k:128 * (k + 1)],
                            gsb[:, 128 * m:128 * (m + 1)],
                            id128,
                        )
                    gts = gtspool.tile([128, 512], F16, tag="gts")
                    nc.vector.tensor_copy(gts[:, :], gt4[:, :])
                    gts_tiles[J8] = gts

                def do_s2(J8):
                    gts = gts_tiles.pop(J8)
                    po = popool.tile([128, 8 * NF], F32, tag="po")
                    x0h = x0a_sb[J8 // 4]
                    for s in range(8):
                        J = 8 * (J8 % 4) + s
                        nc.tensor.matmul(
                            po[:, NF * s:NF * (s + 1)],
                            x0h[:, 128 * J:128 * (J + 1)],
                            gts[:, 128 * (s // 2) + NF * (s % 2):
                                 128 * (s // 2) + NF * (s % 2 + 1)],
                            start=True, stop=True,
                        )
                    o0 = J8 * 8 * NF
                    nc.scalar.copy(osb[:, o0:o0 + 8 * NF], po[:, :])
                    nc.sync.dma_start(out=outd[:, o0:o0 + 8 * NF],
                                      in_=osb[:, o0:o0 + 8 * NF])

                do_tr(0)
                for J8 in range(1, 8):
                    do_tr(J8)
                    do_s2(J8 - 1)
                do_s2(7)
    nc.compile()
    return nc


def _host_prep(x_0, x_k, weight):
    f16 = np.float16
    x_0 = np.asarray(x_0, dtype=np.float32)
    x_k = np.asarray(x_k, dtype=np.float32)
    W = np.asarray(weight, dtype=np.float32).reshape(NF, NI, Y)

    # w4[di*32+y, q*64+n] = W[n, 4q+di, y]
    w4f = W.reshape(NF, 8, 4, Y).transpose(2, 3, 1, 0).reshape(128, 8 * NF)

    iden = np.eye(NF, dtype=np.float32)

    comba_l, xksb_l, x0a_l = [], [], []
    jj = np.arange(4)
    for core in range(NCORES):
        xkc = x_k[core * BPC:(core + 1) * BPC]            # [128, y, f]
        x0c_ = x_0[core * BPC:(core + 1) * BPC]           # [128, t, f]
        # xks4[di*32+y, q*256 + b_l*2 + c] = xk[b_l, y, 2*(4q+di)+c]
        xkr = xkc.reshape(BPC, Y, 8, 4, 2)                # [b_l, y, q, di, c]
        xks4 = xkr.transpose(3, 1, 2, 0, 4).reshape(128, 8 * NBC)
        comba = np.zeros((128, 1664), dtype=np.float32)
        comba[:, 0:512] = w4f
        comba[:, 512:1536] = xks4[:, :4 * NBC]
        comba[:, 1536:1664] = np.eye(128, dtype=np.float32)
        comba_l.append(comba.astype(f16))
        xksb_l.append(np.ascontiguousarray(xks4[:, 4 * NBC:]).astype(f16))
        # x0a[(j,i), (J,j2,t)] = delta(j,j2) * x0bc[4J+j, i, t]
        x0r = x0c_.reshape(BPC, T, NI, 2)                 # [b_l, t, i, c]
        x0bc = x0r.transpose(0, 3, 2, 1).reshape(NBC, NI, T)   # [bc, i, t]
        x0bd = np.zeros((NG, 4, NI, 4, T), dtype=np.float32)
        x0bd[:, jj, :, jj, :] = x0bc.reshape(NG, 4, NI, T).transpose(1, 0, 2, 3)
        # rows ordered (i, j) to match the transposed-G row convention
        x0a = x0bd.transpose(2, 1, 0, 3, 4).reshape(128, NG * 128)
        x0a_l.append(np.ascontiguousarray(x0a).astype(f16))

    return comba_l, xksb_l, x0a_l


def _in_maps(x_0, x_k, weight):
    comba_l, xksb_l, x0a_l = _host_prep(x_0, x_k, weight)
    return [
        {"comba": comba_l[c], "xksb": xksb_l[c], "x0a": x0a_l[c]}
        for c in range(NCORES)
    ]


def kernel(x_0, x_k, weight, bias):
    from concourse import bass_utils

    if "nc" not in _cached:
        _cached["nc"] = _build_bass()
    nc = _cached["nc"]

    in_maps = _in_maps(x_0, x_k, weight)
    res = bass_utils.run_bass_kernel_spmd(nc, in_maps, core_ids=list(range(NCORES)))

    bias = np.asarray(bias, dtype=np.float32)
    outs = []
    for c in range(NCORES):
        od = res.results[c]["outd"].astype(np.float32)  # [128=(j,t), NG*64=(J,n)]
        o = od.reshape(4, T, NG, NF)                # [j, t, J, n]
        o = o.transpose(2, 0, 3, 1)                 # [J, j, n, t]
        o = o.reshape(BPC, 2, NF, T)                # [b_l, c, n, t]
        o = o.transpose(0, 2, 1, 3).reshape(BPC, NF, 2 * T)  # [b_l, n, c*32+t]
        outs.append(o)
    out = np.concatenate(outs, axis=0)
    out = out + bias[None, :, None]
    return np.ascontiguousarray(out.astype(np.float32))


# revision 41
# speedup vs baseline: 1.2483x; 1.2483x over previous
"""Fused CIN-layer kernel for Trainium2 (8 NeuronCores, batch data-parallel).

True reference semantics (derived from the row-major .view + strided conv):
  out[b, n, c*32+t] = sum_{i<32, y<32} W[n,i,y] * x0[b,t,2i+c] * xk[b,y,2i+c] + bias[n]
where c in {0,1} is the f-parity and i indexes f-pairs.

Per core (128 batches, bc = b_local*2 + c in [0,256)):
  warmup:  ~14 dummy matmuls on a zeroed tile while input DMA streams, so
           the PE HAM clock-gate reaches 2.4 GHz before the real work.
  stage1:  row-tiled quads: for quad q, 4 concurrent K=32 matmuls (one per
           i=4q+di, tile_position=(32*di,0)) with lhsT=W_i [32y,64n] and
           rhs=XkS_i [32y,256bc], each into its own PSUM bank of a
           [64, 2048] quad tile.  3-way ACT/DVE/GPSIMD evac into
           gsb[n, bc*32+i] (fp16).
  transpose (PE): per J (4 bc's): Gt_J[(j,i), n] from gsb[n, (j,i)] via PE
           transpose (fp16 PSUM out), DVE-evacuated to SBUF.
  stage2:  out_J[(j,t), n] = x0bd_J^T @ Gt_J where x0bd is a block-diagonal
           x0 tile built ON DEVICE (memset + 4 strided scatter DMAs of the
           compact 512KB x0), loaded as stationary with FWL.
  fp16 output DMA per J8 chunk; host adds bias + final reshape in fp32.
"""

import numpy as np

BS, T, Y, F, NF = 1024, 32, 32, 64, 64
NCORES = 8
BPC = BS // NCORES      # 128 batches per core
NBC = BPC * 2           # 256 (b,c) pairs per core
NG = NBC // 4           # 64 groups of 4
NI = 32                 # f-pair index
NWARM = 8               # HAM warmup matmuls

_cached = {}


def _build_bass():
    import concourse.bass as bass
    import concourse.mybir as mybir
    from concourse import bacc
    from concourse.tile import TileContext

    F16 = mybir.dt.float16
    F32 = mybir.dt.float32

    nc = bacc.Bacc()
    # comba[di*32+y | (n,i)-rows, :]: cols 0:512   w4[., q*64+n]  = W[n,4q+di,y]
    #                                cols 512:1536 xks half0 [., q*256+bc]
    #                                cols 1536:1600 iden (rows 0:64)
    comba = nc.dram_tensor("comba", [128, 1664], F16, kind="ExternalInput")
    # xksb: xks half1 (quads 4..7)
    xksb = nc.dram_tensor("xksb", [128, 4 * NBC], F16, kind="ExternalInput")
    # x0a[(j,i), J*128 + j2*32 + t] = delta(j,j2) * x0bc[4J+j, i, t]
    x0a = nc.dram_tensor("x0a", [128, NG * 128], F16, kind="ExternalInput")
    # outd[(j,t), J*64+n] fp16
    outd = nc.dram_tensor("outd", [128, NG * NF], F16, kind="ExternalOutput")

    with TileContext(nc) as tc:
        with (
            tc.tile_pool(name="const", bufs=1) as cpool,
            tc.tile_pool(name="sb", bufs=1) as spool,
        ):
            wdum = cpool.tile([128, 384], F16)
            nc.vector.memset(wdum[:, :], 0)
            # dependency-free scalar op so ACT_TABLE_LOAD happens during the
            # preamble instead of stalling the first stage-1 evacuation
            scratch = cpool.tile([64, 8], F16)
            nc.scalar.copy(scratch[:, :], wdum[0:64, 0:8])

            # single DMA for everything stage-1 needs (w4 + xks half0 + iden)
            comba_sb = cpool.tile([128, 1664], F16)
            nc.sync.dma_start(out=comba_sb, in_=comba[:, :])
            xksb_sb = cpool.tile([128, 4 * NBC], F16)
            nc.sync.dma_start(out=xksb_sb, in_=xksb[:, :])

            def w4_ap(di, q):
                return comba_sb[32 * di:32 * (di + 1), NF * q:NF * (q + 1)]

            def xks_ap(di, q):
                qc = q % 4
                if q < 4:
                    return comba_sb[32 * di:32 * (di + 1),
                                    512 + NBC * qc:512 + NBC * (qc + 1)]
                return xksb_sb[32 * di:32 * (di + 1), NBC * qc:NBC * (qc + 1)]

            id128 = comba_sb[:, 1536:1664]
            # x0a in two tiles so early s2 rounds aren't gated on the full 2MB
            x0a_sb = [cpool.tile([128, NG * 64], F16, name=f"x0a_sb{h}") for h in range(2)]
            for h in range(2):
                nc.sync.dma_start(out=x0a_sb[h],
                                  in_=x0a[:, NG * 64 * h:NG * 64 * (h + 1)])

            # gsb2[(g,n), m*128 + j*32 + i] = G[n, bc=8m+4g+j, i]
            gsb = spool.tile([128, NBC * NI // 2], F16)
            osb = spool.tile([128, NG * NF], F16)    # out[(j,t), J*64+n]

            # HAM warmup: harmless matmuls on zeros while inputs stream in
            with tc.tile_pool(name="warm", bufs=1, space="PSUM") as wpool:
                wt = wpool.tile([128, 256], F32)
                for _ in range(NWARM):
                    nc.tensor.matmul(wt[:, :], wdum[:, :128], wdum[:, 128:384],
                                     start=True, stop=True)

            # stage 1: per quad q, 8 matmuls (4 row-tiled i's x 2 J-parities).
            # Parity g goes to output partitions [64g, 64g+64) (column tile
            # position 64g) so the psum quad uses all 128 partitions and the
            # evacuations run full-lane.  rhs selects bc's of parity g
            # (stride-8 blocks of 4).
            with tc.tile_pool(name="gq", bufs=4, space="PSUM") as gqpool:
                for q in range(8):
                    gqt = [gqpool.tile([128, 1024], F32, tag="gq",
                                       name=f"gq_{q}_{h}") for h in range(2)]
                    for g in range(2):
                        for h in range(2):
                            for e in range(2):
                                di = 2 * h + e
                                rhs = xks_ap(di, q).rearrange(
                                    "p (m gj) -> p m gj", m=NI, gj=8)[:, :, 4 * g:4 * g + 4]
                                nc.tensor.matmul(
                                    gqt[h][64 * g:64 * (g + 1), 512 * e:512 * e + 128],
                                    w4_ap(di, q),
                                    rhs,
                                    start=True, stop=True,
                                    tile_position=(32 * di, 64 * g),
                                )
                    # evac: gsb2[(g,n), m*128 + i*4 + j] -- m-major keeps each
                    # transpose input one contiguous 128-col run, i*4+j makes
                    # each i-pair evac write 16B bursts
                    for h in range(2):
                        src = gqt[h][:, :].rearrange(
                            "p (e mm j) -> p mm e j",
                            e=2, mm=128, j=4)[:, 0:NI, :, :]
                        dst = gsb[:, :].rearrange(
                            "p (m i j) -> p m i j",
                            m=NI, i=NI, j=4)[:, :, 4 * q + 2 * h:4 * q + 2 * h + 2, :]
                        if h == 0:
                            nc.vector.tensor_copy(dst, src)
                        else:
                            nc.scalar.copy(dst, src)

            # transpose + stage2, software-pipelined per J8 round
            with (
                tc.tile_pool(name="gt", bufs=3, space="PSUM") as gtpool,
                tc.tile_pool(name="po", bufs=3, space="PSUM") as popool,
                tc.tile_pool(name="gts", bufs=3) as gtspool,
            ):
                gts_tiles = {}

                def do_tr(J8):
                    gt4 = gtpool.tile([128, 512], F16, tag="gt8")
                    for k in range(4):
                        m = 4 * J8 + k
                        nc.tensor.transpose(
                            gt4[:, 128 * k:128 * (k + 1)],
                            gsb[:, 128 * m:128 * (m + 1)],
                            id128,
                        )
                    gts = gtspool.tile([128, 512], F16, tag="gts")
                    nc.vector.tensor_copy(gts[:, :], gt4[:, :])
                    gts_tiles[J8] = gts

                def do_s2(J8):
                    gts = gts_tiles.pop(J8)
                    po = popool.tile([128, 8 * NF], F32, tag="po")
                    x0h = x0a_sb[J8 // 4]
                    for s in range(8):
                        J = 8 * (J8 % 4) + s
                        nc.tensor.matmul(
                            po[:, NF * s:NF * (s + 1)],
                            x0h[:, 128 * J:128 * (J + 1)],
                            gts[:, 128 * (s // 2) + NF * (s % 2):
                                 128 * (s // 2) + NF * (s % 2 + 1)],
                            start=True, stop=True,
                        )
                    o0 = J8 * 8 * NF
                    nc.scalar.copy(osb[:, o0:o0 + 8 * NF], po[:, :])
                    if J8 % 2 == 1:
                        d0 = (J8 - 1) * 8 * NF
                        nc.sync.dma_start(out=outd[:, d0:d0 + 16 * NF],
                                          in_=osb[:, d0:d0 + 16 * NF])

                do_tr(0)
                for J8 in range(1, 8):
                    do_tr(J8)
                    do_s2(J8 - 1)
                do_s2(7)
    nc.compile()
    return nc


def _host_prep(x_0, x_k, weight):
    f16 = np.float16
    x_0 = np.asarray(x_0, dtype=np.float32)
    x_k = np.asarray(x_k, dtype=np.float32)
    W = np.asarray(weight, dtype=np.float32).reshape(NF, NI, Y)

    # w4[di*32+y, q*64+n] = W[n, 4q+di, y]
    w4f = W.reshape(NF, 8, 4, Y).transpose(2, 3, 1, 0).reshape(128, 8 * NF)

    iden = np.eye(NF, dtype=np.float32)

    comba_l, xksb_l, x0a_l = [], [], []
    jj = np.arange(4)
    for core in range(NCORES):
        xkc = x_k[core * BPC:(core + 1) * BPC]            # [128, y, f]
        x0c_ = x_0[core * BPC:(core + 1) * BPC]           # [128, t, f]
        # xks4[di*32+y, q*256 + b_l*2 + c] = xk[b_l, y, 2*(4q+di)+c]
        xkr = xkc.reshape(BPC, Y, 8, 4, 2)                # [b_l, y, q, di, c]
        xks4 = xkr.transpose(3, 1, 2, 0, 4).reshape(128, 8 * NBC)
        comba = np.zeros((128, 1664), dtype=np.float32)
        comba[:, 0:512] = w4f
        comba[:, 512:1536] = xks4[:, :4 * NBC]
        comba[:, 1536:1664] = np.eye(128, dtype=np.float32)
        comba_l.append(comba.astype(f16))
        xksb_l.append(np.ascontiguousarray(xks4[:, 4 * NBC:]).astype(f16))
        # x0a[(j,i), (J,j2,t)] = delta(j,j2) * x0bc[4J+j, i, t]
        x0r = x0c_.reshape(BPC, T, NI, 2)                 # [b_l, t, i, c]
        x0bc = x0r.transpose(0, 3, 2, 1).reshape(NBC, NI, T)   # [bc, i, t]
        x0bd = np.zeros((NG, 4, NI, 4, T), dtype=np.float32)
        x0bd[:, jj, :, jj, :] = x0bc.reshape(NG, 4, NI, T).transpose(1, 0, 2, 3)
        # rows ordered (i, j) to match the transposed-G row convention
        x0a = x0bd.transpose(2, 1, 0, 3, 4).reshape(128, NG * 128)
        x0a_l.append(np.ascontiguousarray(x0a).astype(f16))

    return comba_l, xksb_l, x0a_l


def _in_maps(x_0, x_k, weight):
    comba_l, xksb_l, x0a_l = _host_prep(x_0, x_k, weight)
    return [
        {"comba": comba_l[c], "xksb": xksb_l[c], "x0a": x0a_l[c]}
        for c in range(NCORES)
    ]


def kernel(x_0, x_k, weight, bias):
    from concourse import bass_utils

    if "nc" not in _cached:
        _cached["nc"] = _build_bass()
    nc = _cached["nc"]

    in_maps = _in_maps(x_0, x_k, weight)
    res = bass_utils.run_bass_kernel_spmd(nc, in_maps, core_ids=list(range(NCORES)))

    bias = np.asarray(bias, dtype=np.float32)
    outs = []
    for c in range(NCORES):
        od = res.results[c]["outd"].astype(np.float32)  # [128=(j,t), NG*64=(J,n)]
        o = od.reshape(4, T, NG, NF)                # [j, t, J, n]
        o = o.transpose(2, 0, 3, 1)                 # [J, j, n, t]
        o = o.reshape(BPC, 2, NF, T)                # [b_l, c, n, t]
        o = o.transpose(0, 2, 1, 3).reshape(BPC, NF, 2 * T)  # [b_l, n, c*32+t]
        outs.append(o)
    out = np.concatenate(outs, axis=0)
    out = out + bias[None, :, None]
    return np.ascontiguousarray(out.astype(np.float32))


# revision 42
# speedup vs baseline: 1.2989x; 1.0405x over previous
"""Fused CIN-layer kernel for Trainium2 (8 NeuronCores, batch data-parallel).

True reference semantics (derived from the row-major .view + strided conv):
  out[b, n, c*32+t] = sum_{i<32, y<32} W[n,i,y] * x0[b,t,2i+c] * xk[b,y,2i+c] + bias[n]
where c in {0,1} is the f-parity and i indexes f-pairs.

Per core (128 batches, bc = b_local*2 + c in [0,256)):
  warmup:  ~14 dummy matmuls on a zeroed tile while input DMA streams, so
           the PE HAM clock-gate reaches 2.4 GHz before the real work.
  stage1:  row-tiled quads: for quad q, 4 concurrent K=32 matmuls (one per
           i=4q+di, tile_position=(32*di,0)) with lhsT=W_i [32y,64n] and
           rhs=XkS_i [32y,256bc], each into its own PSUM bank of a
           [64, 2048] quad tile.  3-way ACT/DVE/GPSIMD evac into
           gsb[n, bc*32+i] (fp16).
  transpose (PE): per J (4 bc's): Gt_J[(j,i), n] from gsb[n, (j,i)] via PE
           transpose (fp16 PSUM out), DVE-evacuated to SBUF.
  stage2:  out_J[(j,t), n] = x0bd_J^T @ Gt_J where x0bd is a block-diagonal
           x0 tile built ON DEVICE (memset + 4 strided scatter DMAs of the
           compact 512KB x0), loaded as stationary with FWL.
  fp16 output DMA per J8 chunk; host adds bias + final reshape in fp32.
"""

import numpy as np

BS, T, Y, F, NF = 1024, 32, 32, 64, 64
NCORES = 8
BPC = BS // NCORES      # 128 batches per core
NBC = BPC * 2           # 256 (b,c) pairs per core
NG = NBC // 4           # 64 groups of 4
NI = 32                 # f-pair index
NWARM = 0               # HAM warmup matmuls (trs2 is LDW-bound; warmup only delays s1)

_cached = {}


def _build_bass():
    import concourse.bass as bass
    import concourse.mybir as mybir
    from concourse import bacc
    from concourse.tile import TileContext

    F16 = mybir.dt.float16
    F32 = mybir.dt.float32

    nc = bacc.Bacc()
    # comba[di*32+y | (n,i)-rows, :]: cols 0:512   w4[., q*64+n]  = W[n,4q+di,y]
    #                                cols 512:1536 xks half0 [., q*256+bc]
    #                                cols 1536:1600 iden (rows 0:64)
    comba = nc.dram_tensor("comba", [128, 1664], F16, kind="ExternalInput")
    # xksb: xks half1 (quads 4..7)
    xksb = nc.dram_tensor("xksb", [128, 4 * NBC], F16, kind="ExternalInput")
    # x0a[(j,i), J*128 + j2*32 + t] = delta(j,j2) * x0bc[4J+j, i, t]
    x0a = nc.dram_tensor("x0a", [128, NG * 128], F16, kind="ExternalInput")
    # outd[(j,t), J*64+n] fp16
    outd = nc.dram_tensor("outd", [128, NG * NF], F16, kind="ExternalOutput")

    with TileContext(nc) as tc:
        with (
            tc.tile_pool(name="const", bufs=1) as cpool,
            tc.tile_pool(name="sb", bufs=1) as spool,
        ):
            wdum = cpool.tile([128, 384], F16)
            nc.vector.memset(wdum[:, :], 0)
            # dependency-free scalar op so ACT_TABLE_LOAD happens during the
            # preamble instead of stalling the first stage-1 evacuation
            scratch = cpool.tile([64, 8], F16)
            nc.scalar.copy(scratch[:, :], wdum[0:64, 0:8])

            # single DMA for everything stage-1 needs (w4 + xks half0 + iden)
            comba_sb = cpool.tile([128, 1664], F16)
            nc.sync.dma_start(out=comba_sb, in_=comba[:, :])
            xksb_sb = cpool.tile([128, 4 * NBC], F16)
            nc.sync.dma_start(out=xksb_sb, in_=xksb[:, :])

            def w4_ap(di, q):
                return comba_sb[32 * di:32 * (di + 1), NF * q:NF * (q + 1)]

            def xks_ap(di, q):
                qc = q % 4
                if q < 4:
                    return comba_sb[32 * di:32 * (di + 1),
                                    512 + NBC * qc:512 + NBC * (qc + 1)]
                return xksb_sb[32 * di:32 * (di + 1), NBC * qc:NBC * (qc + 1)]

            id128 = comba_sb[:, 1536:1664]
            # x0a in two tiles so early s2 rounds aren't gated on the full 2MB
            x0a_sb = [cpool.tile([128, NG * 64], F16, name=f"x0a_sb{h}") for h in range(2)]
            for h in range(2):
                nc.sync.dma_start(out=x0a_sb[h],
                                  in_=x0a[:, NG * 64 * h:NG * 64 * (h + 1)])

            # gsb2[(g,n), m*128 + j*32 + i] = G[n, bc=8m+4g+j, i]
            gsb = spool.tile([128, NBC * NI // 2], F16)
            osb = spool.tile([128, NG * NF], F16)    # out[(j,t), J*64+n]

            # HAM warmup: harmless matmuls on zeros while inputs stream in
            if NWARM:
                with tc.tile_pool(name="warm", bufs=1, space="PSUM") as wpool:
                    wt = wpool.tile([128, 256], F32)
                    for _ in range(NWARM):
                        nc.tensor.matmul(wt[:, :], wdum[:, :128],
                                         wdum[:, 128:384], start=True, stop=True)

            # stage 1: per quad q, 8 matmuls (4 row-tiled i's x 2 J-parities).
            # Parity g goes to output partitions [64g, 64g+64) (column tile
            # position 64g) so the psum quad uses all 128 partitions and the
            # evacuations run full-lane.  rhs selects bc's of parity g
            # (stride-8 blocks of 4).
            with tc.tile_pool(name="gq", bufs=4, space="PSUM") as gqpool:
                for q in range(8):
                    gqt = [gqpool.tile([128, 1024], F32, tag="gq",
                                       name=f"gq_{q}_{h}") for h in range(2)]
                    for g in range(2):
                        for h in range(2):
                            for e in range(2):
                                di = 2 * h + e
                                rhs = xks_ap(di, q).rearrange(
                                    "p (m gj) -> p m gj", m=NI, gj=8)[:, :, 4 * g:4 * g + 4]
                                nc.tensor.matmul(
                                    gqt[h][64 * g:64 * (g + 1), 512 * e:512 * e + 128],
                                    w4_ap(di, q),
                                    rhs,
                                    start=True, stop=True,
                                    tile_position=(32 * di, 64 * g),
                                )
                    # evac: gsb2[(g,n), m*128 + i*4 + j] -- m-major keeps each
                    # transpose input one contiguous 128-col run, i*4+j makes
                    # each i-pair evac write 16B bursts
                    for h in range(2):
                        src = gqt[h][:, :].rearrange(
                            "p (e mm j) -> p mm e j",
                            e=2, mm=128, j=4)[:, 0:NI, :, :]
                        dst = gsb[:, :].rearrange(
                            "p (m i j) -> p m i j",
                            m=NI, i=NI, j=4)[:, :, 4 * q + 2 * h:4 * q + 2 * h + 2, :]
                        if h == 0:
                            nc.vector.tensor_copy(dst, src)
                        else:
                            nc.scalar.copy(dst, src)

            # transpose + stage2, software-pipelined per J8 round
            with (
                tc.tile_pool(name="gt", bufs=3, space="PSUM") as gtpool,
                tc.tile_pool(name="po", bufs=3, space="PSUM") as popool,
                tc.tile_pool(name="gts", bufs=3) as gtspool,
            ):
                gts_tiles = {}

                def do_tr(J8):
                    gt4 = gtpool.tile([128, 512], F16, tag="gt8")
                    for k in range(4):
                        m = 4 * J8 + k
                        nc.tensor.transpose(
                            gt4[:, 128 * k:128 * (k + 1)],
                            gsb[:, 128 * m:128 * (m + 1)],
                            id128,
                        )
                    gts = gtspool.tile([128, 512], F16, tag="gts")
                    nc.vector.tensor_copy(gts[:, :], gt4[:, :])
                    gts_tiles[J8] = gts

                def do_s2(J8):
                    gts = gts_tiles.pop(J8)
                    po = popool.tile([128, 8 * NF], F32, tag="po")
                    x0h = x0a_sb[J8 // 4]
                    for s in range(8):
                        J = 8 * (J8 % 4) + s
                        nc.tensor.matmul(
                            po[:, NF * s:NF * (s + 1)],
                            x0h[:, 128 * J:128 * (J + 1)],
                            gts[:, 128 * (s // 2) + NF * (s % 2):
                                 128 * (s // 2) + NF * (s % 2 + 1)],
                            start=True, stop=True,
                        )
                    o0 = J8 * 8 * NF
                    if J8 % 2 == 0:
                        nc.scalar.copy(osb[:, o0:o0 + 8 * NF], po[:, :])
                    else:
                        nc.vector.tensor_copy(osb[:, o0:o0 + 8 * NF], po[:, :])
                    # tapered output chunks: big early, small late so the
                    # final transfer doesn't hang off the end
                    if J8 in (1, 3, 5):
                        d0 = (J8 - 1) * 8 * NF
                        nc.sync.dma_start(out=outd[:, d0:d0 + 16 * NF],
                                          in_=osb[:, d0:d0 + 16 * NF])
                    elif J8 in (6, 7):
                        nc.sync.dma_start(out=outd[:, o0:o0 + 8 * NF],
                                          in_=osb[:, o0:o0 + 8 * NF])

                do_tr(0)
                for J8 in range(1, 8):
                    do_tr(J8)
                    do_s2(J8 - 1)
                do_s2(7)
    nc.compile()
    return nc


def _host_prep(x_0, x_k, weight):
    f16 = np.float16
    x_0 = np.asarray(x_0, dtype=np.float32)
    x_k = np.asarray(x_k, dtype=np.float32)
    W = np.asarray(weight, dtype=np.float32).reshape(NF, NI, Y)

    # w4[di*32+y, q*64+n] = W[n, 4q+di, y]
    w4f = W.reshape(NF, 8, 4, Y).transpose(2, 3, 1, 0).reshape(128, 8 * NF)

    iden = np.eye(NF, dtype=np.float32)

    comba_l, xksb_l, x0a_l = [], [], []
    jj = np.arange(4)
    for core in range(NCORES):
        xkc = x_k[core * BPC:(core + 1) * BPC]            # [128, y, f]
        x0c_ = x_0[core * BPC:(core + 1) * BPC]           # [128, t, f]
        # xks4[di*32+y, q*256 + b_l*2 + c] = xk[b_l, y, 2*(4q+di)+c]
        xkr = xkc.reshape(BPC, Y, 8, 4, 2)                # [b_l, y, q, di, c]
        xks4 = xkr.transpose(3, 1, 2, 0, 4).reshape(128, 8 * NBC)
        comba = np.zeros((128, 1664), dtype=np.float32)
        comba[:, 0:512] = w4f
        comba[:, 512:1536] = xks4[:, :4 * NBC]
        comba[:, 1536:1664] = np.eye(128, dtype=np.float32)
        comba_l.append(comba.astype(f16))
        xksb_l.append(np.ascontiguousarray(xks4[:, 4 * NBC:]).astype(f16))
        # x0a[(j,i), (J,j2,t)] = delta(j,j2) * x0bc[4J+j, i, t]
        x0r = x0c_.reshape(BPC, T, NI, 2)                 # [b_l, t, i, c]
        x0bc = x0r.transpose(0, 3, 2, 1).reshape(NBC, NI, T)   # [bc, i, t]
        x0bd = np.zeros((NG, 4, NI, 4, T), dtype=np.float32)
        x0bd[:, jj, :, jj, :] = x0bc.reshape(NG, 4, NI, T).transpose(1, 0, 2, 3)
        # rows ordered (i, j) to match the transposed-G row convention
        x0a = x0bd.transpose(2, 1, 0, 3, 4).reshape(128, NG * 128)
        x0a_l.append(np.ascontiguousarray(x0a).astype(f16))

    return comba_l, xksb_l, x0a_l


def _in_maps(x_0, x_k, weight):
    comba_l, xksb_l, x0a_l = _host_prep(x_0, x_k, weight)
    return [
        {"comba": comba_l[c], "xksb": xksb_l[c], "x0a": x0a_l[c]}
        for c in range(NCORES)
    ]


def kernel(x_0, x_k, weight, bias):
    from concourse import bass_utils

    if "nc" not in _cached:
        _cached["nc"] = _build_bass()
    nc = _cached["nc"]

    in_maps = _in_maps(x_0, x_k, weight)
    res = bass_utils.run_bass_kernel_spmd(nc, in_maps, core_ids=list(range(NCORES)))

    bias = np.asarray(bias, dtype=np.float32)
    outs = []
    for c in range(NCORES):
        od = res.results[c]["outd"].astype(np.float32)  # [128=(j,t), NG*64=(J,n)]
        o = od.reshape(4, T, NG, NF)                # [j, t, J, n]
        o = o.transpose(2, 0, 3, 1)                 # [J, j, n, t]
        o = o.reshape(BPC, 2, NF, T)                # [b_l, c, n, t]
        o = o.transpose(0, 2, 1, 3).reshape(BPC, NF, 2 * T)  # [b_l, n, c*32+t]
        outs.append(o)
    out = np.concatenate(outs, axis=0)
    out = out + bias[None, :, None]
    return np.ascontiguousarray(out.astype(np.float32))
